# revision 21
# baseline (speedup 1.0000x reference)
"""BCH/RS systematic encoder kernel for Trainium2 (8 NeuronCores, data parallel).

Computes out = concat([msg, (msg @ Gp) mod 2], axis=-1) for
msg [16384, 1000] f32 of 0/1 bits and Gp [1000, 256] f32 of 0/1 bits.

Design (per core, 2048 rows, 8 groups of 256 = 2 chunks of 128):
  - SWDGE cast-load msg group f32 -> fp16 SBUF (0/1 exact in fp16)
  - transpose msg [m,k] -> msgT [k,m] per 128x128 block on the PE
    (matmul-with-identity into PSUM fp16, ACT/DVE eviction); 'xbar' mode
    keeps the DMA-crossbar variant for comparison
  - Gp column-packed: Gp2[k,n'] = Gp[k,n'] + 1024*Gp[k,n'+128] (fp16 exact,
    values {0,1,1024,1025}); stationary operand [k=128, n'=128] per k-chunk
  - 8 accumulating matmuls per group: S[n'=128, m=256] += Gp2_kb.T @ msgT_kb
    (f32 PSUM; S <= 1000*1025 < 2^24 so every partial sum is exact)
  - S holds BOTH parity halves: lo = S & 1, hi = (S >> 10) & 1
  - PE f32 transpose of S -> [m, n'], ACT psum->i32, DVE bit extracts,
    cast-copies into the fp16 row tile cols 1000:1128 / 1128:1256
  - SWDGE cast-store fp16 rows -> f32 out [m, 1256], store issues
    interleaved LOOKAHEAD load-groups behind the loads so the HBM write
    stream enters the FIFO ring early and overlaps the read stream
HBM traffic/core = 8.19 MB read + 10.29 MB write (the minimum); the ring
runs at the measured ~410 B/ns per-core HBM cap for the whole window.
"""

import os
import sys

import numpy as np

if os.path.isdir("/opt/trn_rl_repo") and "/opt/trn_rl_repo" not in sys.path:
    sys.path.insert(0, "/opt/trn_rl_repo")

import concourse.bacc as bacc
import concourse.mybir as mybir
import concourse.tile as tile
from concourse.bass_utils import run_bass_kernel_spmd

BATCH = 16384
MSG = 1000
NPAR = 256
NPACK = 128  # packed parity columns (two bits per matmul output value)
NCORES = 8
ROWS = BATCH // NCORES  # 2048
P = 128
KCH = 8  # k chunks; padded K = 1024
KPAD = KCH * P
GC = 2  # m-chunks of 128 per group
GM = GC * P  # 256 rows per group
LOOKAHEAD = 6  # loads queued on the gpsimd ring ahead of the store interleave

# 'pe': msg transpose on the tensor engine; 'xbar': on the DMA crossbar
TMODE = "pe"

# test.py pokes these for profiling
TRACE = False
LAST_RESULT = None

_CACHE = {}


def build_nc(rows=ROWS, tmode=None):
    """Emit the Bass/Tile IR for one core handling `rows` rows."""
    tmode = tmode or TMODE
    n_groups = rows // GM
    assert rows == n_groups * GM
    nc = bacc.Bacc("TRN2", target_bir_lowering=False, debug=False)
    msg = nc.dram_tensor("msg", [rows, MSG], mybir.dt.float32, kind="ExternalInput")
    gp = nc.dram_tensor("gp", [P, KCH * NPACK], mybir.dt.float16, kind="ExternalInput")
    # identity matrices shipped from the host: building them on-device costs
    # gpsimd queue time ahead of the load issues
    idt16 = nc.dram_tensor("idt16", [P, P], mybir.dt.float16, kind="ExternalInput")
    idt32 = nc.dram_tensor("idt32", [P, P], mybir.dt.float32, kind="ExternalInput")
    out = nc.dram_tensor(
        "out", [rows, MSG + NPAR], mybir.dt.float32, kind="ExternalOutput"
    )

    msg3 = msg[:, :].rearrange("(g c p) k -> g c p k", c=GC, p=P)
    out3 = out[:, :].rearrange("(g c p) k -> g c p k", c=GC, p=P)

    # row stride keeps every a[:, c, :] slice 32B-aligned for the xbar
    # transpose: 1264 fp16 = 2528 B = 79*32
    ROWP = 1264

    with tile.TileContext(nc) as tc:
        with (
            tc.tile_pool(name="gpool", bufs=1) as gpool,
            tc.tile_pool(name="ipool", bufs=1) as ipool,
            tc.tile_pool(name="apool", bufs=min(n_groups, 8)) as apool,
            tc.tile_pool(name="bpool", bufs=2) as bpool,
            tc.tile_pool(name="sevpool", bufs=2) as sevpool,
            tc.tile_pool(name="cipool", bufs=2) as cipool,
            tc.tile_pool(name="epool", bufs=2) as epool,
            tc.tile_pool(name="ptpool", bufs=3, space="PSUM") as ptpool,
            tc.tile_pool(name="accpool", bufs=3, space="PSUM") as accpool,
            tc.tile_pool(name="stpool", bufs=2, space="PSUM") as stpool,
        ):
            a_tiles = {}
            b_tiles = {}
            acc_tiles = {}

            def emit_load(g):
                # full output row in fp16: cols 0:1000 msg, 1000:1256 parity
                a = apool.tile([P, GC, ROWP], mybir.dt.float16, tag="a")
                nc.gpsimd.dma_start(
                    out=a[:, :, 0:MSG],
                    in_=msg3[g, :, :, :].rearrange("c p k -> p c k"),
                )
                # zero the transpose pad columns (the pad rows of msgT hit
                # zero Gp2 rows / are zeros streaming into the matmul)
                nc.vector.memset(a[:, :, MSG:KPAD], 0)
                a_tiles[g] = a

            def emit_transpose(g):
                a = a_tiles[g]
                # b[q, c, kb, p] = a[p, c, kb*128+q]  (msgT, k on partitions)
                b = bpool.tile([P, GC, KCH, P], mybir.dt.float16, tag="b")
                if tmode == "xbar":
                    # all on ONE HWDGE ring: concurrent xbar transposes from
                    # two rings corrupt each other
                    for c in range(GC):
                        nc.sync.dma_start(
                            out=b[:, c, :, :], in_=a[:, c, 0:KPAD], transpose=True
                        )
                else:
                    for c in range(GC):
                        pt = ptpool.tile([P, KCH, P], mybir.dt.float16, tag="pt")
                        for kb in range(KCH):
                            nc.tensor.transpose(
                                pt[:, kb, :],
                                a[:, c, kb * P : (kb + 1) * P],
                                ident16[:, :],
                            )
                        # evict msgT chunk PSUM -> SBUF (split ACT/DVE)
                        if c % 2 == 0:
                            nc.scalar.copy(b[:, c, :, :], pt[:, :, :])
                        else:
                            nc.vector.tensor_copy(b[:, c, :, :], pt[:, :, :])
                b_tiles[g] = b

            def emit_mm(g):
                b = b_tiles[g]
                # S[n'=128, m=512] = sum_kb Gp2_kb.T @ msgT_kb  (f32, exact)
                acc = accpool.tile([P, GC * P], mybir.dt.float32, tag="acc")
                accv = acc[:, :].rearrange("q (c p) -> q c p", c=GC)
                for kb in range(KCH):
                    nc.tensor.matmul(
                        accv[:, :, :],
                        gsb[:, kb, :],
                        b[:, :, kb, :],
                        start=(kb == 0),
                        stop=(kb == KCH - 1),
                    )
                acc_tiles[g] = acc

            def emit_post(g):
                a = a_tiles[g]
                acc = acc_tiles.pop(g)
                # evict S to SBUF f32 so the PE can transpose it back
                sev = sevpool.tile([P, GC, P], mybir.dt.float32, tag="sev")
                nc.scalar.copy(sev[:, :, :].rearrange("q c p -> q (c p)"), acc[:, :])
                st = stpool.tile([P, GC, P], mybir.dt.float32, tag="st")
                for c in range(GC):
                    nc.tensor.transpose(st[:, c, :], sev[:, c, :], ident32[:, :])
                # st[p, c, n'] = S[n', c*128+p]; rows (c,p) match a's layout
                ci = cipool.tile([P, GC, NPACK], mybir.dt.int32, tag="ci")
                nc.scalar.copy(ci[:, :, :], st[:, :, :])
                # parity lo = S & 1, hi = (S >> 10) & 1 (bitVec cannot cast)
                elo = epool.tile([P, GC, NPACK], mybir.dt.int32, tag="elo")
                nc.vector.tensor_scalar(
                    elo[:, :, :], ci[:, :, :], 1, None, mybir.AluOpType.bitwise_and
                )
                ehi = epool.tile([P, GC, NPACK], mybir.dt.int32, tag="ehi")
                nc.vector.tensor_scalar(
                    ehi[:, :, :],
                    ci[:, :, :],
                    10,
                    1,
                    mybir.AluOpType.logical_shift_right,
                    mybir.AluOpType.bitwise_and,
                )
                # parity into the output-row tile (0/1 exact in fp16)
                nc.vector.tensor_copy(a[:, :, MSG : MSG + NPACK], elo[:, :, :])
                nc.vector.tensor_copy(a[:, :, MSG + NPACK : MSG + NPAR], ehi[:, :, :])

            def emit_store(g):
                # single cast-store of the full rows: [p, c, 1256] fp16 -> f32.
                # (Splitting msg/parity stores cannot overlap the load stream:
                # one gpsimd ring drains FIFO, and HBM rd+wr share one ~410
                # B/ns cap anyway — measured, no duplex gain.)
                a = a_tiles.pop(g)
                nc.gpsimd.dma_start(
                    out=out3[g, :, :, :].rearrange("c p k -> p c k"),
                    in_=a[:, :, 0 : MSG + NPAR],
                )

            # Gp packed+swizzled resident in SBUF: gsb[q, kb, n'] = Gp2[kb*128+q, n']
            gsb = gpool.tile([P, KCH, NPACK], mybir.dt.float16)
            nc.sync.dma_start(
                out=gsb[:, :, :].rearrange("p a b -> p (a b)"), in_=gp[:, :]
            )
            ident16 = ipool.tile([P, P], mybir.dt.float16, tag="i16")
            nc.sync.dma_start(out=ident16[:, :], in_=idt16[:, :])
            ident32 = ipool.tile([P, P], mybir.dt.float32, tag="i32")
            nc.sync.dma_start(out=ident32[:, :], in_=idt32[:, :])
            # gpsimd ring order IS emission order and drains FIFO, so store
            # descriptors queue behind every load emitted before them. Emit
            # only LOOKAHEAD loads up front, then interleave store g with
            # load g+LOOKAHEAD: the write stream enters the ring ~7us
            # earlier while the queued load backlog guarantees store g's
            # semaphore wait resolves before the ring reaches its slot.
            for g in range(min(LOOKAHEAD, n_groups)):
                emit_load(g)
            # post(g) directly after mm(g): the PE takes a small bubble
            # waiting on the ACT sev eviction, but it has ~16us of slack and
            # every group's store issues ~5us sooner, so the HBM write
            # stream starts right as the load backlog drains
            for it in range(n_groups):
                emit_transpose(it)
                emit_mm(it)
                emit_post(it)
                emit_store(it)
                if it + LOOKAHEAD < n_groups:
                    emit_load(it + LOOKAHEAD)

    nc.compile()
    return nc


def prep_gp(Gp):
    """Pack parity column pairs, pad K to 1024, swizzle to [128, 8*128] fp16."""
    gp = np.asarray(Gp, dtype=np.float32)
    packed = gp[:, :NPACK] + 1024.0 * gp[:, NPACK:]
    gp_pad = np.zeros((KPAD, NPACK), dtype=np.float32)
    gp_pad[:MSG] = packed
    gsw = gp_pad.reshape(KCH, P, NPACK).transpose(1, 0, 2).reshape(P, KCH * NPACK)
    return np.ascontiguousarray(gsw).astype(np.float16)


def kernel(message_bits, Gp):
    global LAST_RESULT
    msg = np.ascontiguousarray(np.asarray(message_bits, dtype=np.float32))
    assert msg.shape == (BATCH, MSG), msg.shape
    gsw = prep_gp(Gp)

    if "nc" not in _CACHE:
        _CACHE["nc"] = build_nc()
    nc = _CACHE["nc"]

    i16 = np.eye(P, dtype=np.float16)
    i32 = np.eye(P, dtype=np.float32)
    in_maps = [
        {"msg": msg[i * ROWS : (i + 1) * ROWS], "gp": gsw, "idt16": i16, "idt32": i32}
        for i in range(NCORES)
    ]
    res = run_bass_kernel_spmd(
        nc, in_maps, core_ids=list(range(NCORES)), trace=TRACE
    )
    LAST_RESULT = res
    return np.concatenate([r["out"] for r in res.results], axis=0)


# revision 23
# speedup vs baseline: 1.1724x; 1.1724x over previous
"""BCH/RS systematic encoder kernel for Trainium2 (8 NeuronCores, data parallel).

Computes out = concat([msg, (msg @ Gp) mod 2], axis=-1) for
msg [16384, 1000] f32 of 0/1 bits and Gp [1000, 256] f32 of 0/1 bits.

Design (per core, 2048 rows, 8 groups of 256 = 2 chunks of 128):
  - SWDGE cast-load msg group f32 -> fp16 SBUF (0/1 exact in fp16)
  - transpose msg [m,k] -> msgT [k,m] per 128x128 block on the PE
    (matmul-with-identity into PSUM fp16, ACT/DVE eviction); 'xbar' mode
    keeps the DMA-crossbar variant for comparison
  - Gp column-packed: Gp2[k,n'] = Gp[k,n'] + 1024*Gp[k,n'+128] (fp16 exact,
    values {0,1,1024,1025}); stationary operand [k=128, n'=128] per k-chunk
  - 8 accumulating matmuls per group: S[n'=128, m=256] += Gp2_kb.T @ msgT_kb
    (f32 PSUM; S <= 1000*1025 < 2^24 so every partial sum is exact)
  - S holds BOTH parity halves: lo = S & 1, hi = (S >> 10) & 1
  - PE f32 transpose of S -> [m, n'], ACT psum->i32, DVE bit extracts,
    cast-copies into the fp16 row tile cols 1000:1128 / 1128:1256
  - SWDGE cast-store fp16 rows -> f32 out [m, 1256], store issues
    interleaved LOOKAHEAD load-groups behind the loads so the HBM write
    stream enters the FIFO ring early and overlaps the read stream
HBM traffic/core = 8.19 MB read + 10.29 MB write (the minimum); the ring
runs at the measured ~410 B/ns per-core HBM cap for the whole window.
"""

import os
import sys

import numpy as np

if os.path.isdir("/opt/trn_rl_repo") and "/opt/trn_rl_repo" not in sys.path:
    sys.path.insert(0, "/opt/trn_rl_repo")

import concourse.bacc as bacc
import concourse.mybir as mybir
import concourse.tile as tile
from concourse.bass_utils import run_bass_kernel_spmd

BATCH = 16384
MSG = 1000
NPAR = 256
NPACK = 128  # packed parity columns (two bits per matmul output value)
NCORES = 8
ROWS = BATCH // NCORES  # 2048
P = 128
KCH = 8  # k chunks; padded K = 1024
KPAD = KCH * P
GC = 2  # m-chunks of 128 per group
GM = GC * P  # 256 rows per group
LOOKAHEAD = 6  # loads queued on the gpsimd ring ahead of the store interleave

# 'pe': msg transpose on the tensor engine; 'xbar': on the DMA crossbar
TMODE = "pe"

# test.py pokes these for profiling
TRACE = False
LAST_RESULT = None

_CACHE = {}


def build_nc(rows=ROWS, tmode=None):
    """Emit the Bass/Tile IR for one core handling `rows` rows."""
    tmode = tmode or TMODE
    n_groups = rows // GM
    assert rows == n_groups * GM
    nc = bacc.Bacc("TRN2", target_bir_lowering=False, debug=False)
    msg = nc.dram_tensor("msg", [rows, MSG], mybir.dt.float32, kind="ExternalInput")
    gp = nc.dram_tensor("gp", [P, KCH * NPACK], mybir.dt.float16, kind="ExternalInput")
    # identity matrices shipped from the host: building them on-device costs
    # gpsimd queue time ahead of the load issues
    idt16 = nc.dram_tensor("idt16", [P, P], mybir.dt.float16, kind="ExternalInput")
    idt32 = nc.dram_tensor("idt32", [P, P], mybir.dt.float32, kind="ExternalInput")
    out = nc.dram_tensor(
        "out", [rows, MSG + NPAR], mybir.dt.float32, kind="ExternalOutput"
    )

    msg3 = msg[:, :].rearrange("(g c p) k -> g c p k", c=GC, p=P)
    out3 = out[:, :].rearrange("(g c p) k -> g c p k", c=GC, p=P)

    # row stride keeps every a[:, c, :] slice 32B-aligned for the xbar
    # transpose: 1264 fp16 = 2528 B = 79*32
    ROWP = 1264

    with tile.TileContext(nc) as tc:
        with (
            tc.tile_pool(name="gpool", bufs=1) as gpool,
            tc.tile_pool(name="ipool", bufs=1) as ipool,
            tc.tile_pool(name="apool", bufs=min(n_groups, 8)) as apool,
            tc.tile_pool(name="bpool", bufs=2) as bpool,
            tc.tile_pool(name="sevpool", bufs=2) as sevpool,
            tc.tile_pool(name="cipool", bufs=2) as cipool,
            tc.tile_pool(name="epool", bufs=2) as epool,
            tc.tile_pool(name="ptpool", bufs=3, space="PSUM") as ptpool,
            tc.tile_pool(name="accpool", bufs=3, space="PSUM") as accpool,
            tc.tile_pool(name="stpool", bufs=2, space="PSUM") as stpool,
        ):
            a_tiles = {}
            b_tiles = {}
            acc_tiles = {}

            def emit_load(g):
                # full output row in fp16: cols 0:1000 msg, 1000:1256 parity
                a = apool.tile([P, GC, ROWP], mybir.dt.float16, tag="a")
                nc.gpsimd.dma_start(
                    out=a[:, :, 0:MSG],
                    in_=msg3[g, :, :, :].rearrange("c p k -> p c k"),
                )
                # zero the transpose pad columns (the pad rows of msgT hit
                # zero Gp2 rows / are zeros streaming into the matmul)
                nc.vector.memset(a[:, :, MSG:KPAD], 0)
                a_tiles[g] = a

            def emit_transpose(g):
                a = a_tiles[g]
                # b[q, c, kb, p] = a[p, c, kb*128+q]  (msgT, k on partitions)
                b = bpool.tile([P, GC, KCH, P], mybir.dt.float16, tag="b")
                if tmode == "xbar":
                    # all on ONE HWDGE ring: concurrent xbar transposes from
                    # two rings corrupt each other
                    for c in range(GC):
                        nc.sync.dma_start(
                            out=b[:, c, :, :], in_=a[:, c, 0:KPAD], transpose=True
                        )
                else:
                    for c in range(GC):
                        pt = ptpool.tile([P, KCH, P], mybir.dt.float16, tag="pt")
                        for kb in range(KCH):
                            nc.tensor.transpose(
                                pt[:, kb, :],
                                a[:, c, kb * P : (kb + 1) * P],
                                ident16[:, :],
                            )
                        # evict msgT chunk PSUM -> SBUF (split ACT/DVE)
                        if c % 2 == 0:
                            nc.scalar.copy(b[:, c, :, :], pt[:, :, :])
                        else:
                            nc.vector.tensor_copy(b[:, c, :, :], pt[:, :, :])
                b_tiles[g] = b

            def emit_mm(g):
                b = b_tiles[g]
                # S[n'=128, m=512] = sum_kb Gp2_kb.T @ msgT_kb  (f32, exact)
                acc = accpool.tile([P, GC * P], mybir.dt.float32, tag="acc")
                accv = acc[:, :].rearrange("q (c p) -> q c p", c=GC)
                for kb in range(KCH):
                    nc.tensor.matmul(
                        accv[:, :, :],
                        gsb[:, kb, :],
                        b[:, :, kb, :],
                        start=(kb == 0),
                        stop=(kb == KCH - 1),
                    )
                acc_tiles[g] = acc

            def emit_post(g):
                a = a_tiles[g]
                acc = acc_tiles.pop(g)
                # evict S to SBUF f32 so the PE can transpose it back
                sev = sevpool.tile([P, GC, P], mybir.dt.float32, tag="sev")
                nc.scalar.copy(sev[:, :, :].rearrange("q c p -> q (c p)"), acc[:, :])
                st = stpool.tile([P, GC, P], mybir.dt.float32, tag="st")
                for c in range(GC):
                    nc.tensor.transpose(st[:, c, :], sev[:, c, :], ident32[:, :])
                # st[p, c, n'] = S[n', c*128+p]; rows (c,p) match a's layout
                ci = cipool.tile([P, GC, NPACK], mybir.dt.int32, tag="ci")
                nc.scalar.copy(ci[:, :, :], st[:, :, :])
                # parity lo = S & 1, hi = (S >> 10) & 1 (bitVec cannot cast)
                elo = epool.tile([P, GC, NPACK], mybir.dt.int32, tag="elo")
                nc.vector.tensor_scalar(
                    elo[:, :, :], ci[:, :, :], 1, None, mybir.AluOpType.bitwise_and
                )
                ehi = epool.tile([P, GC, NPACK], mybir.dt.int32, tag="ehi")
                nc.vector.tensor_scalar(
                    ehi[:, :, :],
                    ci[:, :, :],
                    10,
                    1,
                    mybir.AluOpType.logical_shift_right,
                    mybir.AluOpType.bitwise_and,
                )
                # parity into the output-row tile (0/1 exact in fp16)
                nc.vector.tensor_copy(a[:, :, MSG : MSG + NPACK], elo[:, :, :])
                nc.vector.tensor_copy(a[:, :, MSG + NPACK : MSG + NPAR], ehi[:, :, :])

            def emit_store(g):
                # single cast-store of the full rows: [p, c, 1256] fp16 -> f32.
                # (Splitting msg/parity stores cannot overlap the load stream:
                # one gpsimd ring drains FIFO, and HBM rd+wr share one ~410
                # B/ns cap anyway — measured, no duplex gain.)
                a = a_tiles.pop(g)
                nc.gpsimd.dma_start(
                    out=out3[g, :, :, :].rearrange("c p k -> p c k"),
                    in_=a[:, :, 0 : MSG + NPAR],
                )

            # Gp packed+swizzled resident in SBUF: gsb[q, kb, n'] = Gp2[kb*128+q, n']
            gsb = gpool.tile([P, KCH, NPACK], mybir.dt.float16)
            nc.sync.dma_start(
                out=gsb[:, :, :].rearrange("p a b -> p (a b)"), in_=gp[:, :]
            )
            ident16 = ipool.tile([P, P], mybir.dt.float16, tag="i16")
            nc.sync.dma_start(out=ident16[:, :], in_=idt16[:, :])
            ident32 = ipool.tile([P, P], mybir.dt.float32, tag="i32")
            nc.sync.dma_start(out=ident32[:, :], in_=idt32[:, :])
            # gpsimd ring order IS emission order and drains FIFO, so store
            # descriptors queue behind every load emitted before them. Emit
            # only LOOKAHEAD loads up front, then interleave store g with
            # load g+LOOKAHEAD: the write stream enters the ring ~7us
            # earlier while the queued load backlog guarantees store g's
            # semaphore wait resolves before the ring reaches its slot.
            for g in range(min(LOOKAHEAD, n_groups)):
                emit_load(g)
            # post(g) directly after mm(g): the PE takes a small bubble
            # waiting on the ACT sev eviction, but it has ~16us of slack and
            # every group's store issues ~5us sooner, so the HBM write
            # stream starts right as the load backlog drains
            for it in range(n_groups):
                emit_transpose(it)
                emit_mm(it)
                emit_post(it)
                emit_store(it)
                if it + LOOKAHEAD < n_groups:
                    emit_load(it + LOOKAHEAD)

    nc.compile()
    return nc


def prep_gp(Gp):
    """Pack parity column pairs, pad K to 1024, swizzle to [128, 8*128] fp16."""
    gp = np.asarray(Gp, dtype=np.float32)
    packed = gp[:, :NPACK] + 1024.0 * gp[:, NPACK:]
    gp_pad = np.zeros((KPAD, NPACK), dtype=np.float32)
    gp_pad[:MSG] = packed
    gsw = gp_pad.reshape(KCH, P, NPACK).transpose(1, 0, 2).reshape(P, KCH * NPACK)
    return np.ascontiguousarray(gsw).astype(np.float16)


def kernel(message_bits, Gp):
    global LAST_RESULT
    msg = np.ascontiguousarray(np.asarray(message_bits, dtype=np.float32))
    assert msg.shape == (BATCH, MSG), msg.shape
    gsw = prep_gp(Gp)

    if "nc" not in _CACHE:
        _CACHE["nc"] = build_nc()
    nc = _CACHE["nc"]

    i16 = np.eye(P, dtype=np.float16)
    i32 = np.eye(P, dtype=np.float32)
    in_maps = [
        {"msg": msg[i * ROWS : (i + 1) * ROWS], "gp": gsw, "idt16": i16, "idt32": i32}
        for i in range(NCORES)
    ]
    res = run_bass_kernel_spmd(
        nc, in_maps, core_ids=list(range(NCORES)), trace=TRACE
    )
    LAST_RESULT = res
    return np.concatenate([r["out"] for r in res.results], axis=0)
